# revision 27
# baseline (speedup 1.0000x reference)
"""Trainium2 Bass kernel for DistangledMultiHeadAttention.

Data-parallel over batch B=8 across 8 NeuronCores (one batch element per
core).  All matmul datapaths run in bf16; PSUM accumulation fp32.

Host-side layout prep: q/k/v/adj are transposed and cast to bf16 on the
host (qT/kT/vT [HID, N], adjT [N, N]); Wq is pre-scaled by D**-0.5.

Math (use_adj=1), reduced from the reference: with softmax over j, terms
constant in j drop out, so neither q nor k needs mean-centering:
    X_ref[i,j] = (qh_i - mq)(kh_j - mk) = qh_i.kh_j - mq.kh_j + (j-const)
so  EM[j,i] = exp(qh_i.kh_j - a_j) * adjT,  a_j = mq . kh_j
with mq = mean_i(qh_i).  a is produced per jo-tile by one tiny F=2
matmul (khT slice stationary, -mq pair moving) and fed to the ACT exp as
its per-partition bias.  Remaining pipeline:
    uT  = softmax_N(Wu^T kT + bu)          [H, N]
    AV  = [vh_h | 1s]^T @ EM -> psum [128, i] (rows 64: = s = sum_j EM)
    t2  = (u ⊙ vh)_{pair}^T @ adjT
    rec = 1/s  (ACT copy rows to SBUF base 0, custom-DVE recip in place)
    ATT^T = AV[:64]*rec + t2               -> attT SBUF bf16
    out = ATT @ Wo + bo

The main loop is software-pipelined by emission order: the PE stream for
head pair `mo` has the projection / t2 / bias-matvec work of `mo+1`
woven between each jo-block's score and AV matmuls, so the PE never
waits on the ACT exp / DVE mask latency and its p-state stays at full
clock.  Projection and t2 accumulate in io-split 2KB PSUM tiles (plain
copy evacuations — no global mean needed), freeing a PSUM bank for a
3-deep score pipeline.  Head drains are spread into the following
head's stream; the output projection reuses the shared PSUM pools.
"""

import contextlib
import numpy as np
import sys

for _p in ("/opt/trn_rl_repo",):
    if _p not in sys.path:
        sys.path.insert(0, _p)

import ml_dtypes
import concourse.bass as bass
import concourse.mybir as mybir
import concourse.tile as tile
from concourse import bacc
from concourse.masks import make_identity

FP32 = mybir.dt.float32
BF16 = mybir.dt.bfloat16
AF = mybir.ActivationFunctionType
ALU = mybir.AluOpType
P = 128
N, HID, H, D = 1024, 1024, 16, 64
HD = H * D
KO = HID // P
NO = N // P
MO = HD // P
FREE = 512
NIO = N // FREE
HPP = P // D
GS = 4


def build_core_kernel(use_adj=True):
    """Build the single-core Bass program (SPMD: same program on 8 cores)."""
    nc = bacc.Bacc("TRN2", target_bir_lowering=False, debug=False)

    qT_d = nc.dram_tensor("qT", [HID, N], BF16, kind="ExternalInput")
    kT_d = nc.dram_tensor("kT", [HID, N], BF16, kind="ExternalInput")
    vT_d = nc.dram_tensor("vT", [HID, N], BF16, kind="ExternalInput")
    adjT_d = nc.dram_tensor("adjT", [N, N], BF16, kind="ExternalInput")
    Wq_d = nc.dram_tensor("Wq", [HID, HD], BF16, kind="ExternalInput")
    Wk_d = nc.dram_tensor("Wk", [HID, HD], BF16, kind="ExternalInput")
    Wv_d = nc.dram_tensor("Wv", [HID, HD], BF16, kind="ExternalInput")
    Wu_d = nc.dram_tensor("Wu", [P, KO, H], BF16, kind="ExternalInput")
    Wo_d = nc.dram_tensor("Wo", [HD, HID], BF16, kind="ExternalInput")
    bv_d = nc.dram_tensor("bv", [HD], BF16, kind="ExternalInput")
    bu_d = nc.dram_tensor("bu", [H], FP32, kind="ExternalInput")
    bo_d = nc.dram_tensor("bo", [HID], BF16, kind="ExternalInput")
    out_d = nc.dram_tensor("out", [N, HID], FP32, kind="ExternalOutput")

    with tile.TileContext(nc) as tc:
        with (
            tc.tile_pool(name="persist", bufs=1) as pp,
            tc.tile_pool(name="small", bufs=1) as sp,
            tc.tile_pool(name="meanp", bufs=4) as meanp,
            tc.tile_pool(name="wkq0", bufs=1) as wkq0,
        ):
            qhT = pp.tile([P, MO, N], BF16, tag="qhT")
            khT = pp.tile([P, MO, N], BF16, tag="khT")
            # AV stationary per (jo, h): [vh_h | ones*64] — psum rows 64:128
            # all receive s = sum_j EM (no partition broadcast needed).
            vha = pp.tile([P, NO, H, 2 * D], BF16, tag="vha")
            attT = pp.tile([P, MO, N], BF16, tag="attT")
            WoSB = pp.tile([P, MO, HID], BF16, tag="WoSB")
            kT = pp.tile([P, KO, N], BF16, tag="kT")
            qT = pp.tile([P, KO, N], BF16, tag="qT")
            if use_adj:
                adjT = pp.tile([P, NO, N], BF16, tag="adjT")
                t2sb = pp.tile([P, MO, N], BF16, tag="t2sb")

            bv_bc = sp.tile([P, HD], BF16, tag="bv")
            bo_bc = sp.tile([P, HID], BF16, tag="bo")
            bu_sb = sp.tile([H, 1], FP32, tag="bu")
            unaryT = sp.tile([H, N], BF16, tag="unaryT")
            uT = sp.tile([H, N], BF16, tag="uT")
            u_nat = sp.tile([P, NO, H], BF16, tag="u_nat")
            ident = sp.tile([P, P], BF16, tag="ident")

            for ko in range(KO):
                nc.sync.dma_start(kT[:, ko, :], kT_d[ko * P:(ko + 1) * P, :])
            nc.sync.dma_start(bu_sb[:], bu_d[:, None])
            Wu_early = sp.tile([P, KO, H], BF16, tag="Wu")
            nc.sync.dma_start(Wu_early[:], Wu_d[:])
            nc.sync.dma_start(bv_bc[:], bv_d[None, :].to_broadcast((P, HD)))
            nc.sync.dma_start(bo_bc[:], bo_d[None, :].to_broadcast((P, HID)))
            make_identity(nc, ident[:])
            # ones columns of vh_aug (rows 64:128 of AV psum = s); written
            # before anything else touches vha — disjoint from the vh cols.
            nc.vector.memset(vha[:, :, :, D:], 1.0)
            # mo=0 weight slices, hoisted so the first projections never
            # wait on DMA at the prefix -> main transition.
            Wk0 = wkq0.tile([P, KO, P], BF16, tag="wk0")
            Wq0 = wkq0.tile([P, KO, P], BF16, tag="wq0")
            nc.sync.dma_start(
                Wk0[:], Wk_d[:, 0:P].rearrange("(ko p) f -> p ko f", p=P))
            nc.sync.dma_start(
                Wq0[:], Wq_d[:, 0:P].rearrange("(ko p) f -> p ko f", p=P))

            with (
                tc.tile_pool(name="bigp", bufs=3, space="PSUM") as bigp,
                tc.tile_pool(name="xps", bufs=2, space="PSUM") as xps,
            ):
                # ---------- emission helpers (thunk lists) ----------------
                def proj_chunks(xT, W_d, mo, dst, reduce_mu, W_pre=None):
                    """8 thunks (2 matmuls each); last emits the evac.

                    reduce_mu: also produce -mean_i of the projection into
                    st['nmu'] (q path; drives the score exp bias)."""
                    st = {}

                    def mm(ko):
                        if ko == 0:
                            if W_pre is None:
                                W_sb = wkq.tile([P, KO, P], BF16, tag="wkq",
                                                name="W_sb")
                                nc.sync.dma_start(
                                    W_sb[:],
                                    W_d[:, mo * P:(mo + 1) * P]
                                    .rearrange("(ko p) f -> p ko f", p=P))
                                st["W"] = W_sb
                            else:
                                st["W"] = W_pre
                            st["ps"] = bigp.tile([P, N], FP32, tag="bp",
                                                 name="ps")
                        for io in range(NIO):
                            nc.tensor.matmul(
                                st["ps"][:, io * FREE:(io + 1) * FREE],
                                st["W"][:, ko, :],
                                xT[:, ko, io * FREE:(io + 1) * FREE],
                                start=(ko == 0), stop=(ko == KO - 1))
                        if ko == KO - 1:
                            ps = st["ps"]
                            nc.vector.tensor_copy(dst[:, mo, :], ps[:])
                            if reduce_mu:
                                nmu = meanp.tile([P, 1], FP32, tag="nmu",
                                                 name="nmu")
                                nc.vector.tensor_reduce(
                                    nmu[:], ps[:], mybir.AxisListType.X,
                                    ALU.add)
                                nc.vector.tensor_scalar(
                                    nmu[:], nmu[:], -1.0 / N, None,
                                    op0=ALU.mult)
                                st["nmu"] = nmu

                    return [(lambda ko=ko: mm(ko)) for ko in range(KO)], st

                def abias_emit(mo, qst, ps, st):
                    """a_nat[:, jo, hp] = -mq_h . khT_h[:, jo-tile]: 8 tiny
                    F=2 matmuls into cols [0:16] of `ps` (the t2 slot, before
                    its accumulation starts), then one evac copy."""
                    mu2 = meanp.tile([P, HPP], BF16, tag="mu2", name="mu2")
                    nc.vector.memset(mu2[:], 0.0)
                    nmu = qst["nmu"]
                    for hp in range(HPP):
                        nc.vector.tensor_copy(
                            mu2[hp * D:(hp + 1) * D, hp, None],
                            nmu[hp * D:(hp + 1) * D, :])
                    for jo in range(NO):
                        nc.tensor.matmul(
                            ps[:, jo * HPP:(jo + 1) * HPP],
                            khT[:, mo, jo * P:(jo + 1) * P],
                            mu2[:], start=True, stop=True)
                    ab = abp.tile([P, NO, HPP], FP32, tag="ab", name="ab")
                    nc.vector.tensor_copy(
                        ab[:],
                        ps[:, :NO * HPP]
                        .rearrange("p (no hp) -> p no hp", hp=HPP))
                    st["ab"] = ab

                def t2_chunks(mo, qst, ast):
                    """Thunks: thunk0 = abias + wt staging; then 8 x (2
                    matmuls); last emits the ACT evacs."""
                    st = {"wts": []}

                    def stage():
                        st["ps"] = bigp.tile([P, N], FP32, tag="bp",
                                             name="pb")
                        abias_emit(mo, qst, st["ps"], ast)
                        for jo in range(NO):
                            wt = wtp.tile([P, HPP, D], BF16, tag="wt",
                                          name="wt")
                            nc.vector.tensor_tensor(
                                wt[:],
                                vha[:, jo, mo * HPP:(mo + 1) * HPP, 0:D],
                                u_nat[:, jo, mo * HPP:(mo + 1) * HPP,
                                      None].to_broadcast((P, HPP, D)),
                                ALU.mult)
                            st["wts"].append(wt)

                    def mm(jo):
                        if jo == 0:
                            stage()
                        for io in range(NIO):
                            nc.tensor.matmul(
                                st["ps"][:, io * FREE:(io + 1) * FREE],
                                st["wts"][jo][:]
                                .rearrange("p h d -> p (h d)"),
                                adjT[:, jo, io * FREE:(io + 1) * FREE],
                                start=(jo == 0), stop=(jo == NO - 1))
                        if jo == NO - 1:
                            for io in range(NIO):
                                nc.scalar.activation(
                                    t2sb[:, mo, io * FREE:(io + 1) * FREE],
                                    st["ps"][:, io * FREE:(io + 1) * FREE],
                                    AF.Copy)

                    return [(lambda jo=jo: mm(jo)) for jo in range(NO)]

                # ---------- per-head emission pieces ----------------------
                def head_scores(h, jo, ems, ab):
                    mo, hp = h // HPP, h % HPP
                    em = emp.tile([P, N], BF16, tag="em", name="em")
                    for io in range(NIO):
                        xp = xps.tile([P, FREE], FP32, tag="xp", name="xp")
                        nc.tensor.matmul(
                            xp[:],
                            khT[hp * D:(hp + 1) * D, mo, jo * P:(jo + 1) * P],
                            qhT[hp * D:(hp + 1) * D, mo,
                                io * FREE:(io + 1) * FREE],
                            start=True, stop=True)
                        nc.scalar.activation(
                            em[:, io * FREE:(io + 1) * FREE], xp[:], AF.Exp,
                            bias=ab[:, jo, hp, None])
                        if use_adj:
                            nc.vector.tensor_tensor(
                                em[:, io * FREE:(io + 1) * FREE],
                                em[:, io * FREE:(io + 1) * FREE],
                                adjT[:, jo, io * FREE:(io + 1) * FREE],
                                ALU.mult)
                    ems[jo] = em

                def head_av(h, jo, ems, st):
                    if jo == 0:
                        st["pa"] = bigp.tile([P, N], FP32, tag="bp",
                                             name="pa")
                    for io in range(NIO):
                        nc.tensor.matmul(
                            st["pa"][:, io * FREE:(io + 1) * FREE],
                            vha[:, jo, h, :],
                            ems[jo][:, io * FREE:(io + 1) * FREE],
                            start=(jo == 0), stop=(jo == NO - 1))

                def head_drain_thunks(h, st):
                    mo, hp = h // HPP, h % HPP
                    dst = {}

                    def c0():
                        rec = recp.tile([D, N], FP32, tag="rec", name="rec")
                        nc.vector.tensor_copy(rec[:], st["pa"][D:2 * D, :])
                        dst["rec"] = rec

                    def c1():
                        nc.vector.reciprocal_approx_fast(dst["rec"][:],
                                                         dst["rec"][:])

                    def c2():
                        att = attT[hp * D:(hp + 1) * D, mo, :]
                        nc.vector.tensor_tensor(att, st["pa"][0:D, :],
                                                dst["rec"][:], ALU.mult)

                    def c3():
                        att = attT[hp * D:(hp + 1) * D, mo, :]
                        if use_adj:
                            nc.vector.tensor_tensor(
                                att, att, t2sb[hp * D:(hp + 1) * D, mo, :],
                                ALU.add)
                        else:
                            t2 = sp.tile([D, 1], FP32, tag=f"t2_{h % 4}",
                                         name="t2")
                            pb1 = xps.tile([P, FREE], FP32, tag="xp",
                                           name="pb1")
                            for jo in range(NO):
                                nc.tensor.matmul(
                                    pb1[0:D, 0:1], vha[:, jo, h, 0:D],
                                    u_nat[:, jo, h, None],
                                    start=(jo == 0), stop=(jo == NO - 1))
                            nc.vector.tensor_copy(t2[:], pb1[0:D, 0:1])
                            nc.vector.tensor_tensor(
                                att, att, t2[:].to_broadcast((D, N)),
                                ALU.add)

                    return [c0, c1, c2, c3]

                # ---------- prefix ----------------------------------------
                with (
                    tc.tile_pool(name="prefix", bufs=1) as pfx,
                    tc.tile_pool(name="wup", bufs=1) as wup,
                ):
                    vT = pfx.tile([P, KO, N], BF16, tag="vT")
                    Wv_sb = pfx.tile([P, KO, HD], BF16, tag="Wv")
                    Wu_sb = Wu_early
                    for ko in range(KO):
                        nc.sync.dma_start(vT[:, ko, :],
                                          vT_d[ko * P:(ko + 1) * P, :])
                        nc.sync.dma_start(Wv_sb[:, ko, :],
                                          Wv_d[ko * P:(ko + 1) * P, :])
                    if use_adj:
                        for no in range(NO):
                            nc.sync.dma_start(adjT[:, no, :],
                                              adjT_d[no * P:(no + 1) * P, :])
                    for ko in range(KO):
                        nc.sync.dma_start(qT[:, ko, :],
                                          qT_d[ko * P:(ko + 1) * P, :])
                    nc.sync.dma_start(
                        WoSB[:], Wo_d[:].rearrange("(mo p) f -> p mo f", p=P))

                    # unary potential u (PE -> ACT -> DVE)
                    for io in range(NIO):
                        up = xps.tile([P, FREE], FP32, tag="xp", name="up")
                        for ko in range(KO):
                            nc.tensor.matmul(
                                up[0:H, :], Wu_sb[:, ko, :],
                                kT[:, ko, io * FREE:(io + 1) * FREE],
                                start=(ko == 0), stop=(ko == KO - 1))
                        nc.scalar.activation(
                            unaryT[:, io * FREE:(io + 1) * FREE],
                            up[0:H, :], AF.Identity, bias=bu_sb[:])
                    usum = sp.tile([H, 1], FP32, tag="usum")
                    urec = sp.tile([H, 1], FP32, tag="urec")
                    nc.scalar.activation(uT[:], unaryT[:], AF.Exp,
                                         accum_out=usum[:])
                    nc.vector.reciprocal(urec[:], usum[:])
                    nc.vector.tensor_scalar(uT[:], uT[:], urec[:], None,
                                            op0=ALU.mult)

                    # v projection: vha = v @ Wv + bv (natural layout)
                    for nb in range(NO):
                        ps = bigp.tile([P, N], FP32, tag="bp", name="vps")
                        for ko in range(KO):
                            for mf in range(NIO):
                                nc.tensor.matmul(
                                    ps[:, mf * FREE:(mf + 1) * FREE],
                                    vT[:, ko, nb * P:(nb + 1) * P],
                                    Wv_sb[:, ko, mf * FREE:(mf + 1) * FREE],
                                    start=(ko == 0), stop=(ko == KO - 1))
                        for mf in range(NIO):
                            hh = mf * (H // NIO)
                            nc.vector.tensor_tensor(
                                vha[:, nb, hh:hh + H // NIO, 0:D],
                                ps[:, mf * FREE:(mf + 1) * FREE]
                                .rearrange("p (h d) -> p h d", d=D),
                                bv_bc[:, mf * FREE:(mf + 1) * FREE]
                                .rearrange("p (h d) -> p h d", d=D),
                                ALU.add)

                    # u_nat [128, NO, H] via PE transposes of uT
                    for g in range(NO // GS):
                        tp = xps.tile([P, FREE], BF16, tag="xp", name="tpn")
                        for t in range(GS):
                            no = g * GS + t
                            nc.tensor.transpose(
                                tp[:, t * H:(t + 1) * H],
                                uT[:, no * P:(no + 1) * P], ident[:H, :H])
                        nc.scalar.activation(
                            u_nat[:, g * GS:(g + 1) * GS, :],
                            tp[:, :GS * H].rearrange("p (g h) -> p g h",
                                                     g=GS),
                            AF.Copy)

                # ---------- software-pipelined main loop -------------------
                _es = contextlib.ExitStack()
                wkq = _es.enter_context(tc.tile_pool(name="wkq", bufs=4))
                emp = _es.enter_context(tc.tile_pool(name="emp", bufs=4))
                recp = _es.enter_context(tc.tile_pool(name="recp", bufs=2))
                wtp = _es.enter_context(tc.tile_pool(name="wtp", bufs=2 * NO))
                abp = _es.enter_context(tc.tile_pool(name="abp", bufs=2))

                # mo=0 proj + t2(+a-bias) emitted plainly first
                thk, _ = proj_chunks(kT, Wk_d, 0, khT, False, W_pre=Wk0)
                for th in thk:
                    th()
                thq, qst0 = proj_chunks(qT, Wq_d, 0, qhT, True, W_pre=Wq0)
                for th in thq:
                    th()
                ast = {}
                for th in t2_chunks(0, qst0, ast):
                    th()

                pending = []          # drain thunks from the previous head

                def pop_pending(k):
                    while k > 0 and pending:
                        pending.pop(0)()
                        k -= 1

                for mo in range(MO):
                    fill = []
                    next_ast = {}
                    if mo + 1 < MO:
                        thk, _ = proj_chunks(kT, Wk_d, mo + 1, khT, False)
                        fill += thk
                        thq, qst = proj_chunks(qT, Wq_d, mo + 1, qhT, True)
                        fill += thq
                        fill += t2_chunks(mo + 1, qst, next_ast)
                    fidx = [0]

                    def F(n):
                        while n > 0 and fidx[0] < len(fill):
                            fill[fidx[0]]()
                            fidx[0] += 1
                            n -= 1

                    ab = ast["ab"]
                    for hp in range(HPP):
                        h = HPP * mo + hp
                        ems, st = {}, {}
                        head_scores(h, 0, ems, ab)
                        pop_pending(2)
                        for jo in range(NO):
                            if jo + 1 < NO:
                                head_scores(h, jo + 1, ems, ab)
                            F(3 if jo % 2 == 0 else 2)
                            pop_pending(1)
                            head_av(h, jo, ems, st)
                        pending.extend(head_drain_thunks(h, st))
                    F(len(fill))      # flush leftover fillers
                    ast = next_ast

                pop_pending(len(pending))

                # ---------- output projection ------------------------------
                with tc.tile_pool(name="outp", bufs=3) as outp:
                    for ic in range(NO):
                        op = bigp.tile([P, N], FP32, tag="bp", name="op")
                        for mo in range(MO):
                            for mf in range(NIO):
                                nc.tensor.matmul(
                                    op[:, mf * FREE:(mf + 1) * FREE],
                                    attT[:, mo, ic * P:(ic + 1) * P],
                                    WoSB[:, mo, mf * FREE:(mf + 1) * FREE],
                                    start=(mo == 0), stop=(mo == MO - 1))
                        for mf in range(NIO):
                            outt = outp.tile([P, FREE], FP32, tag="outt",
                                             name="outt")
                            nc.vector.tensor_tensor(
                                outt[:],
                                op[:, mf * FREE:(mf + 1) * FREE],
                                bo_bc[:, mf * FREE:(mf + 1) * FREE],
                                ALU.add)
                            nc.sync.dma_start(
                                out_d[ic * P:(ic + 1) * P,
                                      mf * FREE:(mf + 1) * FREE], outt[:])
                _es.close()

    nc.compile()
    return nc


_CACHE = {}


def _get_nc(use_adj: bool):
    key = bool(use_adj)
    if key not in _CACHE:
        _CACHE[key] = build_core_kernel(use_adj=key)
    return _CACHE[key]


def _make_in_maps(ins=None, **kw):
    if ins is None:
        ins = kw
    BF = ml_dtypes.bfloat16
    gf = lambda n: np.ascontiguousarray(np.asarray(ins[n], np.float32))
    gb = lambda n: gf(n).astype(BF)
    q = np.asarray(ins["q"], np.float32)
    k = np.asarray(ins["k"], np.float32)
    v = np.asarray(ins["v"], np.float32)
    adj = np.asarray(ins["adj"], np.float32)
    qT = np.ascontiguousarray(q.transpose(0, 2, 1)).astype(BF)
    kT = np.ascontiguousarray(k.transpose(0, 2, 1)).astype(BF)
    vT = np.ascontiguousarray(v.transpose(0, 2, 1)).astype(BF)
    adjT = np.ascontiguousarray(adj.transpose(0, 2, 1)).astype(BF)
    Wu = np.asarray(ins["Wu"], np.float32)
    scale = float(D) ** (-0.5)
    shared = {
        "Wq": (gf("Wq") * scale).astype(BF),
        "Wk": gb("Wk"), "Wv": gb("Wv"), "Wo": gb("Wo"),
        "Wu": np.ascontiguousarray(
            Wu.reshape(KO, P, H).transpose(1, 0, 2)).astype(BF),
        "bv": gb("bv"), "bu": gf("bu"), "bo": gb("bo"),
    }
    in_maps = []
    for b in range(q.shape[0]):
        m = dict(shared)
        m["qT"], m["kT"], m["vT"], m["adjT"] = qT[b], kT[b], vT[b], adjT[b]
        in_maps.append(m)
    return in_maps


def kernel(q, k, v, adj, use_adj, Wq, bq, Wk, bk, Wv, bv, Wu, bu, Wo, bo):
    from concourse.bass_utils import run_bass_kernel_spmd

    nc = _get_nc(bool(int(np.asarray(use_adj))))
    in_maps = _make_in_maps(q=q, k=k, v=v, adj=adj, Wq=Wq, Wk=Wk, Wv=Wv,
                            Wu=Wu, Wo=Wo, bv=bv, bu=bu, bo=bo)
    res = run_bass_kernel_spmd(nc, in_maps, list(range(len(in_maps))))
    return np.stack([res.results[b]["out"] for b in range(len(in_maps))],
                    axis=0)


# revision 29
# speedup vs baseline: 1.0871x; 1.0871x over previous
"""Trainium2 Bass kernel for DistangledMultiHeadAttention.

Data-parallel over batch B=8 across 8 NeuronCores (one batch element per
core).  All matmul datapaths run in bf16; PSUM accumulation fp32.

Host-side layout prep: q/k/v/adj are transposed and cast to bf16 on the
host (qT/kT/vT [HID, N], adjT [N, N]); Wq is pre-scaled by D**-0.5.

Math (use_adj=1), reduced from the reference: with softmax over j, terms
constant in j drop out, so neither q nor k needs mean-centering:
    X_ref[i,j] = (qh_i - mq)(kh_j - mk) = qh_i.kh_j - mq.kh_j + (j-const)
so  EM[j,i] = exp(qh_i.kh_j - a_j) * adjT,  a_j = mq . kh_j
with mq = mean_i(qh_i).  a is produced per jo-tile by one tiny F=2
matmul (khT slice stationary, -mq pair moving) and fed to the ACT exp as
its per-partition bias.  Remaining pipeline:
    uT  = softmax_N(Wu^T kT + bu)          [H, N]
    AV  = [vh_h | 1s]^T @ EM -> psum [128, i] (rows 64: = s = sum_j EM)
    t2  = (u ⊙ vh)_{pair}^T @ adjT
    rec = 1/s  (ACT copy rows to SBUF base 0, custom-DVE recip in place)
    ATT^T = AV[:64]*rec + t2               -> attT SBUF bf16
    out = ATT @ Wo + bo

The main loop is software-pipelined by emission order: the PE stream for
head pair `mo` has the projection / t2 / bias-matvec work of `mo+1`
woven between each jo-block's score and AV matmuls, so the PE never
waits on the ACT exp / DVE mask latency and its p-state stays at full
clock.  Projection and t2 accumulate in io-split 2KB PSUM tiles (plain
copy evacuations — no global mean needed), freeing a PSUM bank for a
3-deep score pipeline.  Head drains are spread into the following
head's stream; the output projection reuses the shared PSUM pools.
"""

import contextlib
import numpy as np
import sys

for _p in ("/opt/trn_rl_repo",):
    if _p not in sys.path:
        sys.path.insert(0, _p)

import ml_dtypes
import concourse.bass as bass
import concourse.mybir as mybir
import concourse.tile as tile
from concourse import bacc
from concourse.masks import make_identity

FP32 = mybir.dt.float32
BF16 = mybir.dt.bfloat16
AF = mybir.ActivationFunctionType
ALU = mybir.AluOpType
P = 128
N, HID, H, D = 1024, 1024, 16, 64
HD = H * D
KO = HID // P
NO = N // P
MO = HD // P
FREE = 512
NIO = N // FREE
HPP = P // D
GS = 4


def build_core_kernel(use_adj=True):
    """Build the single-core Bass program (SPMD: same program on 8 cores)."""
    nc = bacc.Bacc("TRN2", target_bir_lowering=False, debug=False)

    qT_d = nc.dram_tensor("qT", [HID, N], BF16, kind="ExternalInput")
    kT_d = nc.dram_tensor("kT", [HID, N], BF16, kind="ExternalInput")
    vT_d = nc.dram_tensor("vT", [HID, N], BF16, kind="ExternalInput")
    adjT_d = nc.dram_tensor("adjT", [N, N], BF16, kind="ExternalInput")
    Wq_d = nc.dram_tensor("Wq", [HID, HD], BF16, kind="ExternalInput")
    Wk_d = nc.dram_tensor("Wk", [HID, HD], BF16, kind="ExternalInput")
    Wv_d = nc.dram_tensor("Wv", [HID, HD], BF16, kind="ExternalInput")
    Wu_d = nc.dram_tensor("Wu", [P, KO, H], BF16, kind="ExternalInput")
    Wo_d = nc.dram_tensor("Wo", [HD, HID], BF16, kind="ExternalInput")
    bv_d = nc.dram_tensor("bv", [HD], BF16, kind="ExternalInput")
    bu_d = nc.dram_tensor("bu", [H], FP32, kind="ExternalInput")
    bo_d = nc.dram_tensor("bo", [HID], BF16, kind="ExternalInput")
    out_d = nc.dram_tensor("out", [N, HID], FP32, kind="ExternalOutput")

    with tile.TileContext(nc) as tc:
        with (
            tc.tile_pool(name="persist", bufs=1) as pp,
            tc.tile_pool(name="small", bufs=1) as sp,
            tc.tile_pool(name="meanp", bufs=4) as meanp,
            tc.tile_pool(name="wkq0", bufs=1) as wkq0,
        ):
            qhT = pp.tile([P, MO, N], BF16, tag="qhT")
            khT = pp.tile([P, MO, N], BF16, tag="khT")
            # AV stationary per (jo, h): [vh_h | ones*64] — psum rows 64:128
            # all receive s = sum_j EM (no partition broadcast needed).
            vha = pp.tile([P, NO, H, 2 * D], BF16, tag="vha")
            attT = pp.tile([P, MO, N], BF16, tag="attT")
            WoSB = pp.tile([P, MO, HID], BF16, tag="WoSB")
            kT = pp.tile([P, KO, N], BF16, tag="kT")
            qT = pp.tile([P, KO, N], BF16, tag="qT")
            if use_adj:
                adjT = pp.tile([P, NO, N], BF16, tag="adjT")
                t2sb = pp.tile([P, MO, N], BF16, tag="t2sb")

            bv_bc = sp.tile([P, HD], BF16, tag="bv")
            bo_bc = sp.tile([P, HID], BF16, tag="bo")
            bu_sb = sp.tile([H, 1], FP32, tag="bu")
            unaryT = sp.tile([H, N], BF16, tag="unaryT")
            uT = sp.tile([H, N], BF16, tag="uT")
            u_nat = sp.tile([P, NO, H], BF16, tag="u_nat")
            ident = sp.tile([P, P], BF16, tag="ident")

            for ko in range(KO):
                nc.sync.dma_start(kT[:, ko, :], kT_d[ko * P:(ko + 1) * P, :])
            nc.sync.dma_start(bu_sb[:], bu_d[:, None])
            Wu_early = sp.tile([P, KO, H], BF16, tag="Wu")
            nc.sync.dma_start(Wu_early[:], Wu_d[:])
            nc.sync.dma_start(bv_bc[:], bv_d[None, :].to_broadcast((P, HD)))
            nc.sync.dma_start(bo_bc[:], bo_d[None, :].to_broadcast((P, HID)))
            make_identity(nc, ident[:])
            # ones columns of vh_aug (rows 64:128 of AV psum = s); written
            # before anything else touches vha — disjoint from the vh cols.
            nc.vector.memset(vha[:, :, :, D:], 1.0)
            # mo=0 weight slices, hoisted so the first projections never
            # wait on DMA at the prefix -> main transition.
            Wk0 = wkq0.tile([P, KO, P], BF16, tag="wk0")
            Wq0 = wkq0.tile([P, KO, P], BF16, tag="wq0")
            nc.sync.dma_start(
                Wk0[:], Wk_d[:, 0:P].rearrange("(ko p) f -> p ko f", p=P))
            nc.sync.dma_start(
                Wq0[:], Wq_d[:, 0:P].rearrange("(ko p) f -> p ko f", p=P))

            with (
                tc.tile_pool(name="bigp", bufs=3, space="PSUM") as bigp,
                tc.tile_pool(name="xps", bufs=2, space="PSUM") as xps,
            ):
                # ---------- emission helpers (thunk lists) ----------------
                def proj_chunks(xT, W_d, mo, dst, reduce_mu, W_pre=None):
                    """8 thunks (2 matmuls each); last emits the evac.

                    reduce_mu: also produce -mean_i of the projection into
                    st['nmu'] (q path; drives the score exp bias)."""
                    st = {}

                    def mm(ko):
                        if ko == 0:
                            if W_pre is None:
                                W_sb = wkq.tile([P, KO, P], BF16, tag="wkq",
                                                name="W_sb")
                                nc.sync.dma_start(
                                    W_sb[:],
                                    W_d[:, mo * P:(mo + 1) * P]
                                    .rearrange("(ko p) f -> p ko f", p=P))
                                st["W"] = W_sb
                            else:
                                st["W"] = W_pre
                            st["ps"] = bigp.tile([P, N], FP32, tag="bp",
                                                 name="ps")
                        for io in range(NIO):
                            nc.tensor.matmul(
                                st["ps"][:, io * FREE:(io + 1) * FREE],
                                st["W"][:, ko, :],
                                xT[:, ko, io * FREE:(io + 1) * FREE],
                                start=(ko == 0), stop=(ko == KO - 1))
                        if ko == KO - 1:
                            ps = st["ps"]
                            if reduce_mu:
                                # center q: qc = qh - mean_i(qh) (k stays
                                # uncentered; its term is softmax-invariant)
                                nmu = meanp.tile([P, 1], FP32, tag="nmu",
                                                 name="nmu")
                                nc.vector.tensor_reduce(
                                    nmu[:], ps[:], mybir.AxisListType.X,
                                    ALU.add)
                                nc.vector.tensor_scalar(
                                    nmu[:], nmu[:], 1.0 / N, None,
                                    op0=ALU.mult)
                                nc.vector.tensor_scalar(
                                    dst[:, mo, :], ps[:], nmu[:], None,
                                    op0=ALU.subtract)
                            else:
                                nc.vector.tensor_copy(dst[:, mo, :], ps[:])

                    return [(lambda ko=ko: mm(ko)) for ko in range(KO)], st

                def t2_chunks(mo):
                    """Thunks: thunk0 = wt staging; then 8 x (2 matmuls);
                    last emits the ACT evacs."""
                    st = {"wts": []}

                    def stage():
                        st["ps"] = bigp.tile([P, N], FP32, tag="bp",
                                             name="pb")
                        for jo in range(NO):
                            wt = wtp.tile([P, HPP, D], BF16, tag="wt",
                                          name="wt")
                            nc.vector.tensor_tensor(
                                wt[:],
                                vha[:, jo, mo * HPP:(mo + 1) * HPP, 0:D],
                                u_nat[:, jo, mo * HPP:(mo + 1) * HPP,
                                      None].to_broadcast((P, HPP, D)),
                                ALU.mult)
                            st["wts"].append(wt)

                    def mm(jo):
                        if jo == 0:
                            stage()
                        for io in range(NIO):
                            nc.tensor.matmul(
                                st["ps"][:, io * FREE:(io + 1) * FREE],
                                st["wts"][jo][:]
                                .rearrange("p h d -> p (h d)"),
                                adjT[:, jo, io * FREE:(io + 1) * FREE],
                                start=(jo == 0), stop=(jo == NO - 1))
                        if jo == NO - 1:
                            for io in range(NIO):
                                nc.scalar.activation(
                                    t2sb[:, mo, io * FREE:(io + 1) * FREE],
                                    st["ps"][:, io * FREE:(io + 1) * FREE],
                                    AF.Copy)

                    return [(lambda jo=jo: mm(jo)) for jo in range(NO)]

                # ---------- per-head emission pieces ----------------------
                def head_scores(h, jo, ems):
                    mo, hp = h // HPP, h % HPP
                    em = emp.tile([P, N], BF16, tag="em", name="em")
                    for io in range(NIO):
                        xp = xps.tile([P, FREE], FP32, tag="xp", name="xp")
                        nc.tensor.matmul(
                            xp[:],
                            khT[hp * D:(hp + 1) * D, mo, jo * P:(jo + 1) * P],
                            qhT[hp * D:(hp + 1) * D, mo,
                                io * FREE:(io + 1) * FREE],
                            start=True, stop=True)
                        nc.scalar.activation(
                            em[:, io * FREE:(io + 1) * FREE], xp[:], AF.Exp)
                        if use_adj:
                            nc.vector.tensor_tensor(
                                em[:, io * FREE:(io + 1) * FREE],
                                em[:, io * FREE:(io + 1) * FREE],
                                adjT[:, jo, io * FREE:(io + 1) * FREE],
                                ALU.mult)
                    ems[jo] = em

                def head_av(h, jo, ems, st):
                    if jo == 0:
                        st["pa"] = bigp.tile([P, N], FP32, tag="bp",
                                             name="pa")
                    for io in range(NIO):
                        nc.tensor.matmul(
                            st["pa"][:, io * FREE:(io + 1) * FREE],
                            vha[:, jo, h, :],
                            ems[jo][:, io * FREE:(io + 1) * FREE],
                            start=(jo == 0), stop=(jo == NO - 1))

                def head_drain_thunks(h, st):
                    mo, hp = h // HPP, h % HPP
                    dst = {}

                    def c0():
                        rec = recp.tile([D, N], FP32, tag="rec", name="rec")
                        nc.vector.tensor_copy(rec[:], st["pa"][D:2 * D, :])
                        dst["rec"] = rec

                    def c1():
                        nc.vector.reciprocal_approx_fast(dst["rec"][:],
                                                         dst["rec"][:])

                    def c2():
                        att = attT[hp * D:(hp + 1) * D, mo, :]
                        nc.vector.tensor_tensor(att, st["pa"][0:D, :],
                                                dst["rec"][:], ALU.mult)

                    def c3():
                        att = attT[hp * D:(hp + 1) * D, mo, :]
                        if use_adj:
                            nc.vector.tensor_tensor(
                                att, att, t2sb[hp * D:(hp + 1) * D, mo, :],
                                ALU.add)
                        else:
                            t2 = sp.tile([D, 1], FP32, tag=f"t2_{h % 4}",
                                         name="t2")
                            pb1 = xps.tile([P, FREE], FP32, tag="xp",
                                           name="pb1")
                            for jo in range(NO):
                                nc.tensor.matmul(
                                    pb1[0:D, 0:1], vha[:, jo, h, 0:D],
                                    u_nat[:, jo, h, None],
                                    start=(jo == 0), stop=(jo == NO - 1))
                            nc.vector.tensor_copy(t2[:], pb1[0:D, 0:1])
                            nc.vector.tensor_tensor(
                                att, att, t2[:].to_broadcast((D, N)),
                                ALU.add)

                    return [c0, c1, c2, c3]

                # ---------- prefix ----------------------------------------
                with (
                    tc.tile_pool(name="prefix", bufs=1) as pfx,
                    tc.tile_pool(name="wup", bufs=1) as wup,
                ):
                    vT = pfx.tile([P, KO, N], BF16, tag="vT")
                    Wv_sb = pfx.tile([P, KO, HD], BF16, tag="Wv")
                    Wu_sb = Wu_early
                    for ko in range(KO):
                        nc.sync.dma_start(vT[:, ko, :],
                                          vT_d[ko * P:(ko + 1) * P, :])
                        nc.sync.dma_start(Wv_sb[:, ko, :],
                                          Wv_d[ko * P:(ko + 1) * P, :])
                    if use_adj:
                        for no in range(NO):
                            nc.sync.dma_start(adjT[:, no, :],
                                              adjT_d[no * P:(no + 1) * P, :])
                    for ko in range(KO):
                        nc.sync.dma_start(qT[:, ko, :],
                                          qT_d[ko * P:(ko + 1) * P, :])
                    nc.sync.dma_start(
                        WoSB[:], Wo_d[:].rearrange("(mo p) f -> p mo f", p=P))

                    # unary potential u (PE -> ACT -> DVE)
                    for io in range(NIO):
                        up = xps.tile([P, FREE], FP32, tag="xp", name="up")
                        for ko in range(KO):
                            nc.tensor.matmul(
                                up[0:H, :], Wu_sb[:, ko, :],
                                kT[:, ko, io * FREE:(io + 1) * FREE],
                                start=(ko == 0), stop=(ko == KO - 1))
                        nc.scalar.activation(
                            unaryT[:, io * FREE:(io + 1) * FREE],
                            up[0:H, :], AF.Identity, bias=bu_sb[:])
                    usum = sp.tile([H, 1], FP32, tag="usum")
                    urec = sp.tile([H, 1], FP32, tag="urec")
                    nc.scalar.activation(uT[:], unaryT[:], AF.Exp,
                                         accum_out=usum[:])
                    nc.vector.reciprocal(urec[:], usum[:])
                    nc.vector.tensor_scalar(uT[:], uT[:], urec[:], None,
                                            op0=ALU.mult)

                    # v projection: vha = v @ Wv + bv (natural layout)
                    for nb in range(NO):
                        ps = bigp.tile([P, N], FP32, tag="bp", name="vps")
                        for ko in range(KO):
                            for mf in range(NIO):
                                nc.tensor.matmul(
                                    ps[:, mf * FREE:(mf + 1) * FREE],
                                    vT[:, ko, nb * P:(nb + 1) * P],
                                    Wv_sb[:, ko, mf * FREE:(mf + 1) * FREE],
                                    start=(ko == 0), stop=(ko == KO - 1))
                        for mf in range(NIO):
                            hh = mf * (H // NIO)
                            nc.vector.tensor_tensor(
                                vha[:, nb, hh:hh + H // NIO, 0:D],
                                ps[:, mf * FREE:(mf + 1) * FREE]
                                .rearrange("p (h d) -> p h d", d=D),
                                bv_bc[:, mf * FREE:(mf + 1) * FREE]
                                .rearrange("p (h d) -> p h d", d=D),
                                ALU.add)

                    # u_nat [128, NO, H] via PE transposes of uT
                    for g in range(NO // GS):
                        tp = xps.tile([P, FREE], BF16, tag="xp", name="tpn")
                        for t in range(GS):
                            no = g * GS + t
                            nc.tensor.transpose(
                                tp[:, t * H:(t + 1) * H],
                                uT[:, no * P:(no + 1) * P], ident[:H, :H])
                        nc.scalar.activation(
                            u_nat[:, g * GS:(g + 1) * GS, :],
                            tp[:, :GS * H].rearrange("p (g h) -> p g h",
                                                     g=GS),
                            AF.Copy)

                # ---------- software-pipelined main loop -------------------
                _es = contextlib.ExitStack()
                wkq = _es.enter_context(tc.tile_pool(name="wkq", bufs=4))
                emp = _es.enter_context(tc.tile_pool(name="emp", bufs=4))
                recp = _es.enter_context(tc.tile_pool(name="recp", bufs=2))
                wtp = _es.enter_context(tc.tile_pool(name="wtp", bufs=2 * NO))

                # mo=0 proj + t2 emitted plainly first
                thk, _ = proj_chunks(kT, Wk_d, 0, khT, False, W_pre=Wk0)
                for th in thk:
                    th()
                thq, _ = proj_chunks(qT, Wq_d, 0, qhT, True, W_pre=Wq0)
                for th in thq:
                    th()
                for th in t2_chunks(0):
                    th()

                pending = []          # drain thunks from finished heads

                def pop_pending(k):
                    while k > 0 and pending:
                        pending.pop(0)()
                        k -= 1

                # One continuous block stream over all (mo, hp, jo): scores
                # run one block ahead of AV across head and mo boundaries
                # alike, so there are no boundary stalls to reset the PE
                # p-state.  Fillers (next mo's proj/t2, and for the last mo
                # the ic=0 out-projection prefix) weave between them.
                blocks = [(mo, HPP * mo + hp, jo)
                          for mo in range(MO)
                          for hp in range(HPP)
                          for jo in range(NO)]
                NB = len(blocks)
                ems_by_h = {}
                st_by_h = {}
                fill, fidx = [], [0]
                op0 = {}

                def F(n):
                    while n > 0 and fidx[0] < len(fill):
                        fill[fidx[0]]()
                        fidx[0] += 1
                        n -= 1

                def emit_scores(t):
                    mo, h, jo = blocks[t]
                    ems = ems_by_h.setdefault(h, {})
                    head_scores(h, jo, ems)

                def out0_chunks():
                    # out-projection ic=0, contraction over mo=0..6 only —
                    # legal during mo=7's heads; finished in the tail.
                    st = {}

                    def mm(mo):
                        if mo == 0:
                            st["op"] = bigp.tile([P, N], FP32, tag="bp",
                                                 name="op")
                            op0["op"] = st["op"]
                        for mf in range(NIO):
                            nc.tensor.matmul(
                                st["op"][:, mf * FREE:(mf + 1) * FREE],
                                attT[:, mo, 0:P],
                                WoSB[:, mo, mf * FREE:(mf + 1) * FREE],
                                start=(mo == 0), stop=False)

                    return [(lambda mo=mo: mm(mo)) for mo in range(MO - 1)]

                emit_scores(0)
                for t in range(NB):
                    mo, h, jo = blocks[t]
                    if jo == 0 and h % HPP == 0:
                        # entering a new mo: queue next mo's filler work
                        fill, fidx = [], [0]
                        if mo + 1 < MO:
                            thk, _ = proj_chunks(kT, Wk_d, mo + 1, khT,
                                                 False)
                            fill += thk
                            thq, _ = proj_chunks(qT, Wq_d, mo + 1, qhT,
                                                 True)
                            fill += thq
                            fill += t2_chunks(mo + 1)
                        else:
                            fill += out0_chunks()
                    if t + 1 < NB:
                        emit_scores(t + 1)
                    F(2 if t % 2 == 0 else 1)
                    pop_pending(1)
                    head_av(h, jo, ems_by_h[h], st_by_h.setdefault(h, {}))
                    if jo == NO - 1:
                        pending.extend(head_drain_thunks(h, st_by_h[h]))
                        F(2)
                F(len(fill))
                pop_pending(len(pending))

                # ---------- output projection ------------------------------
                with tc.tile_pool(name="outp", bufs=3) as outp:
                    for ic in range(NO):
                        if ic == 0 and "op" in op0:
                            op = op0["op"]
                            mo_range = [MO - 1]
                        else:
                            op = bigp.tile([P, N], FP32, tag="bp", name="op")
                            mo_range = list(range(MO))
                        for mo in mo_range:
                            for mf in range(NIO):
                                nc.tensor.matmul(
                                    op[:, mf * FREE:(mf + 1) * FREE],
                                    attT[:, mo, ic * P:(ic + 1) * P],
                                    WoSB[:, mo, mf * FREE:(mf + 1) * FREE],
                                    start=(mo == 0), stop=(mo == MO - 1))
                        for mf in range(NIO):
                            outt = outp.tile([P, FREE], FP32, tag="outt",
                                             name="outt")
                            nc.vector.tensor_tensor(
                                outt[:],
                                op[:, mf * FREE:(mf + 1) * FREE],
                                bo_bc[:, mf * FREE:(mf + 1) * FREE],
                                ALU.add)
                            nc.sync.dma_start(
                                out_d[ic * P:(ic + 1) * P,
                                      mf * FREE:(mf + 1) * FREE], outt[:])
                _es.close()

    nc.compile()
    return nc


_CACHE = {}


def _get_nc(use_adj: bool):
    key = bool(use_adj)
    if key not in _CACHE:
        _CACHE[key] = build_core_kernel(use_adj=key)
    return _CACHE[key]


def _make_in_maps(ins=None, **kw):
    if ins is None:
        ins = kw
    BF = ml_dtypes.bfloat16
    gf = lambda n: np.ascontiguousarray(np.asarray(ins[n], np.float32))
    gb = lambda n: gf(n).astype(BF)
    q = np.asarray(ins["q"], np.float32)
    k = np.asarray(ins["k"], np.float32)
    v = np.asarray(ins["v"], np.float32)
    adj = np.asarray(ins["adj"], np.float32)
    qT = np.ascontiguousarray(q.transpose(0, 2, 1)).astype(BF)
    kT = np.ascontiguousarray(k.transpose(0, 2, 1)).astype(BF)
    vT = np.ascontiguousarray(v.transpose(0, 2, 1)).astype(BF)
    adjT = np.ascontiguousarray(adj.transpose(0, 2, 1)).astype(BF)
    Wu = np.asarray(ins["Wu"], np.float32)
    scale = float(D) ** (-0.5)
    shared = {
        "Wq": (gf("Wq") * scale).astype(BF),
        "Wk": gb("Wk"), "Wv": gb("Wv"), "Wo": gb("Wo"),
        "Wu": np.ascontiguousarray(
            Wu.reshape(KO, P, H).transpose(1, 0, 2)).astype(BF),
        "bv": gb("bv"), "bu": gf("bu"), "bo": gb("bo"),
    }
    in_maps = []
    for b in range(q.shape[0]):
        m = dict(shared)
        m["qT"], m["kT"], m["vT"], m["adjT"] = qT[b], kT[b], vT[b], adjT[b]
        in_maps.append(m)
    return in_maps


def kernel(q, k, v, adj, use_adj, Wq, bq, Wk, bk, Wv, bv, Wu, bu, Wo, bo):
    from concourse.bass_utils import run_bass_kernel_spmd

    nc = _get_nc(bool(int(np.asarray(use_adj))))
    in_maps = _make_in_maps(q=q, k=k, v=v, adj=adj, Wq=Wq, Wk=Wk, Wv=Wv,
                            Wu=Wu, Wo=Wo, bv=bv, bu=bu, bo=bo)
    res = run_bass_kernel_spmd(nc, in_maps, list(range(len(in_maps))))
    return np.stack([res.results[b]["out"] for b in range(len(in_maps))],
                    axis=0)
